# revision 1
# baseline (speedup 1.0000x reference)
"""GCN layer (x @ W.T aggregated over sparse adjacency) on 8 TRN2 NeuronCores.

Math:  out[d,:] = sum_{e: row[e]==d} val[e] * (x[col[e],:] @ W.T) + bias
Equivalently (used here): out = (A @ x) @ W.T + bias   with A the sparse
edge_val-weighted adjacency.  Aggregating raw x first avoids materializing
the dense `support` table: x itself (cast to bf16) is the gather table.

Sharding: destination nodes are split across the 8 cores (6250 each); each
core receives the full x table (replicated, free at exec time) plus its own
edge metadata, gathers source rows with dma_gather (int16 indices -> the
table is split at 32768 into lo/hi halves with rebased indices), and
segment-sums into its output rows via PE matmuls with per-chunk scaled
selector matrices.  Output rows are written back with plain DMAs and
concatenated on the host.

Gathers are pad-inclusive: the padded per-(tile,half) segments are laid out
contiguously, pad slots carry index 0 / val 0 / dest -1, and each (batch,
half) region is fetched with 1024-index dma_gather calls rotated over all 4
SWDGE queues.  Pad rows gather row 0 redundantly but are nulled by the
selector matmuls, so no memsets are needed.  Queue rotation keeps each
call's decode-time ring-space wait off the previous call's drain (the ring
holds ~1024 descriptor rows per direction; a 64-row call pipelines ~15
deep), and Q7 descriptor generation (~2.5 ns/idx, serial on the Pool
engine) sets the gather floor at ~250-300 us/core for the ~108k rows.
"""

import math

import numpy as np
import ml_dtypes

import concourse.bacc as bacc
import concourse.mybir as mybir
import concourse.tile as tile
from concourse.bass_utils import run_bass_kernel_spmd

# Problem constants (fixed by the harness).
N_NODES = 50000
N_EDGES = 800000
D = 128
C = 8                      # cores
NPC = N_NODES // C         # 6250 destination nodes per core
P = 128
T = math.ceil(NPC / P)     # 49 dest tiles per core
SPLIT = 32768              # int16-index split point of the gather table
N_LO = SPLIT
N_HI = N_NODES - SPLIT
G_TILES = 4                # dest tiles per gather batch
CALL = 1536                # indices per dma_gather call (ring-pipelining sweet spot)

BF16 = mybir.dt.bfloat16
F32 = mybir.dt.float32
I16 = mybir.dt.int16


def _ru(x, m):
    return (x + m - 1) // m * m


def _prep_host(edge_row, edge_col, edge_val):
    """Partition/sort/pad edges.  Returns the shared static structure plus
    per-core flat arrays (idx, dest-in-tile, val) in the padded layout."""
    er = np.asarray(edge_row).astype(np.int64)
    ec = np.asarray(edge_col).astype(np.int64)
    ev = np.asarray(edge_val).astype(np.float32)

    core = er // NPC
    dloc = er % NPC
    tl = dloc // P
    din = dloc % P
    half = (ec >= SPLIT).astype(np.int64)

    # counts per (core, tile, half)
    cnt = np.zeros((C, T, 2), np.int64)
    np.add.at(cnt, (core, tl, half), 1)

    # shared sizes (max over cores), 128-rounded chunk structure.  Pad slots
    # carry idx 0 / dtv -1 / val 0: gathered redundantly, nulled in matmuls.
    n_pad = np.zeros((T, 2), np.int64)
    for t in range(T):
        for h in range(2):
            n_pad[t, h] = _ru(max(int(cnt[:, t, h].max()), 1), P)

    # batches of dest tiles; within a batch the layout is
    # [lo seg of t0 | lo of t1 | ... | hi of t0 | hi of t1 | ...]
    batches = [list(range(b, min(b + G_TILES, T))) for b in range(0, T, G_TILES)]
    seg_off = np.zeros((T, 2), np.int64)  # flat offset of each (tile, half) segment
    call_off = []                         # per batch: (lo_off, lo_len, hi_off, hi_len)
    off = 0
    for bt in batches:
        lo_off = off
        for t in bt:
            seg_off[t, 0] = off
            off += n_pad[t, 0]
        lo_len = off - lo_off
        hi_off = off
        for t in bt:
            seg_off[t, 1] = off
            off += n_pad[t, 1]
        call_off.append((lo_off, lo_len, hi_off, off - hi_off))
    L = off                               # total padded edges per core
    K = L // P                            # total 128-edge chunks

    # flat padded position of every edge:  seg_off[tile,half] + rank in
    # segment.  Edges within a segment are sorted by source id so gather
    # descriptors hit ascending HBM addresses (better DRAM locality).
    order = np.lexsort((ec, half, tl, core))
    so = seg_off[tl[order], half[order]]
    # rank within each (core,tile,half) group (groups are contiguous in `order`)
    key = (core[order] * T + tl[order]) * 2 + half[order]
    newgrp = np.ones(len(key), bool)
    newgrp[1:] = key[1:] != key[:-1]
    idxs = np.arange(len(key))
    grp_start = np.maximum.accumulate(np.where(newgrp, idxs, 0))
    rank = idxs - grp_start
    pos = so + rank                      # padded flat position, per `order` entry

    idx_flat = np.zeros((C, L), np.int16)     # pad -> row 0 (weight 0 kills it)
    din_flat = np.full((C, L), -1.0, np.float32)
    val_flat = np.zeros((C, L), np.float32)
    oc = core[order]
    rebased = np.where(half[order] == 1, ec[order] - SPLIT, ec[order])
    idx_flat[oc, pos] = rebased.astype(np.int16)
    din_flat[oc, pos] = din[order].astype(np.float32)
    val_flat[oc, pos] = ev[order]

    return dict(
        n_pad=n_pad, batches=batches, seg_off=seg_off,
        call_off=call_off, L=L, K=K, idx_flat=idx_flat, din_flat=din_flat,
        val_flat=val_flat,
    )


def _wrap_idx(idx_flat_core):
    """Build the [128, L//16] int16 index tensor: flat index j lives at
    partition j%16 (replicated to all 8 groups of 16 partitions), free
    column j//16.  Valid for any dma_gather call window at a multiple of
    16 because all call offsets here are multiples of 128."""
    L = idx_flat_core.shape[0]
    out = np.zeros((P, L // 16), np.int16)
    out[:16, :] = idx_flat_core.reshape(L // 16, 16).T
    for g in range(1, 8):
        out[16 * g:16 * (g + 1), :] = out[:16, :]
    return out


def _build_program(st, repeat=1, skip_gather=False, skip_compute=False,
                   n_queues=4, single_packet=False, call=CALL, msgs_bufs=2,
                   dds=16384, slack=0):
    """Emit the Bass/Tile program (shared by all 8 cores).  repeat>1 wraps
    the main loop in a For_i for wall-clock timing amplification.
    skip_gather/skip_compute build partial variants for HW profiling."""
    n_pad, batches, seg_off, call_off = (
        st["n_pad"], st["batches"], st["seg_off"], st["call_off"])
    L, K = st["L"], st["K"]

    nc = bacc.Bacc("TRN2", target_bir_lowering=False,
                   num_swdge_queues=n_queues,
                   dynamic_dma_scratch_size=dds)
    x_lo = nc.dram_tensor("x_lo", [N_LO, D], BF16, kind="ExternalInput")
    x_hi = nc.dram_tensor("x_hi", [N_HI, D], BF16, kind="ExternalInput")
    idx_d = nc.dram_tensor("idx", [P, L // 16], I16, kind="ExternalInput")
    din_d = nc.dram_tensor("din", [P, K], F32, kind="ExternalInput")
    val_d = nc.dram_tensor("val", [P, K], F32, kind="ExternalInput")
    wt_d = nc.dram_tensor("wt", [P, D], BF16, kind="ExternalInput")
    iota_d = nc.dram_tensor("iota", [P, P], BF16, kind="ExternalInput")
    bias_d = nc.dram_tensor("bias_row", [1, D], BF16, kind="ExternalInput")
    ones_d = nc.dram_tensor("ones_row", [1, P], BF16, kind="ExternalInput")
    out_d = nc.dram_tensor("out", [NPC, D], F32, kind="ExternalOutput")

    qctr = [0]

    with tile.TileContext(nc) as tc:
        with (
            tc.tile_pool(name="const", bufs=1) as cpool,
            tc.tile_pool(name="msgs", bufs=msgs_bufs) as mpool,
            tc.tile_pool(name="st", bufs=16 if slack else 8) as spool,
            tc.tile_pool(name="aggp", bufs=6 if slack else 4,
                         space="PSUM") as agg_pool,
            tc.tile_pool(name="outp", bufs=2, space="PSUM") as outp_pool,
            tc.tile_pool(name="aggs", bufs=4 if slack else 3) as aggs_pool,
            tc.tile_pool(name="outs", bufs=3) as outs_pool,
        ):
            idx_sb = cpool.tile([P, L // 16], I16)
            din_sb = cpool.tile([P, K], F32)
            val_sb = cpool.tile([P, K], F32)
            wt_sb = cpool.tile([P, D], BF16)
            iota_sb = cpool.tile([P, P], BF16)
            bias_sb = cpool.tile([1, D], BF16)
            ones_sb = cpool.tile([1, P], BF16)
            nc.sync.dma_start(out=idx_sb[:], in_=idx_d[:])
            nc.sync.dma_start(out=din_sb[:], in_=din_d[:])
            nc.sync.dma_start(out=val_sb[:], in_=val_d[:])
            nc.sync.dma_start(out=wt_sb[:], in_=wt_d[:])
            nc.sync.dma_start(out=iota_sb[:], in_=iota_d[:])
            nc.sync.dma_start(out=bias_sb[:], in_=bias_d[:])
            nc.sync.dma_start(out=ones_sb[:], in_=ones_d[:])

            def body():
                for bi, bt in enumerate(batches):
                    _emit_batch(bi, bt)

            kb_max = max((lo + hi) // P for (_, lo, _, hi) in call_off)

            def _emit_batch(bi, bt):
                lo_off, lo_len, hi_off, hi_len = call_off[bi]
                boff = lo_off           # batch base (flat edges)
                msgs = mpool.tile([P, kb_max, D], BF16, tag="msgs")
                # gather lo + hi halves for the whole batch, pad-inclusive
                if not skip_gather:
                    for o, l, table in ((lo_off, lo_len, x_lo),
                                        (hi_off, hi_len, x_hi)):
                        for so in range(0, l, call):
                            sl = min(call, l - so)
                            c0 = (o + so - boff) // P
                            nch = sl // P
                            nc.gpsimd.dma_gather(
                                out_ap=msgs[:, c0:c0 + nch, :],
                                in_ap=table[:],
                                idxs_ap=idx_sb[:, (o + so) // 16:
                                               (o + so + sl) // 16],
                                num_idxs=sl,
                                num_idxs_reg=sl,
                                elem_size=D,
                                single_packet=single_packet,
                                queue_num=qctr[0] % n_queues,
                            )
                            qctr[0] += 1
                if skip_compute:
                    return
                outs = outs_pool.tile([P, len(bt), D], F32, tag="outs")
                for ti, t in enumerate(bt):
                    kt = int((n_pad[t, 0] + n_pad[t, 1]) // P)
                    aggp = agg_pool.tile([P, P], F32, tag="aggp")
                    j = 0
                    for h in range(2):
                        g0 = int(seg_off[t, h]) // P     # global chunk idx
                        c0 = (int(seg_off[t, h]) - boff) // P  # within batch
                        for q in range(int(n_pad[t, h]) // P):
                            stile = spool.tile([P, P], BF16, tag="st")
                            nc.vector.tensor_scalar(
                                out=stile[:],
                                in0=iota_sb[:],
                                scalar1=din_sb[:, g0 + q:g0 + q + 1],
                                scalar2=val_sb[:, g0 + q:g0 + q + 1],
                                op0=mybir.AluOpType.is_equal,
                                op1=mybir.AluOpType.mult,
                            )
                            nc.tensor.matmul(
                                out=aggp[:],
                                lhsT=msgs[:, c0 + q, :],
                                rhs=stile[:],
                                start=(j == 0),
                                stop=(j == kt - 1),
                            )
                            j += 1
                    # aggp = agg^T [feat x dest]; cast to bf16 and transform
                    aggs = aggs_pool.tile([P, P], BF16, tag="aggs")
                    nc.scalar.copy(out=aggs[:], in_=aggp[:])
                    outp = outp_pool.tile([P, D], F32, tag="outp")
                    nc.tensor.matmul(out=outp[:], lhsT=aggs[:], rhs=wt_sb[:],
                                     start=True, stop=False)
                    nc.tensor.matmul(out=outp[:], lhsT=ones_sb[:],
                                     rhs=bias_sb[:], start=False, stop=True)
                    nc.scalar.copy(out=outs[:, ti, :], in_=outp[:])
                # batched output write: row r of the batch = outs[r%128, r//128]
                r0 = bt[0] * P
                rows = min(NPC, (bt[-1] + 1) * P) - r0
                if rows == len(bt) * P:
                    hbm = out_d[r0:r0 + rows, :].rearrange(
                        "(c p) f -> p c f", p=P)
                    nc.sync.dma_start(out=hbm, in_=outs[:])
                else:
                    nfull = rows // P
                    if nfull:
                        hbm = out_d[r0:r0 + nfull * P, :].rearrange(
                            "(c p) f -> p c f", p=P)
                        nc.sync.dma_start(out=hbm, in_=outs[:, :nfull, :])
                    rem = rows - nfull * P
                    if rem:
                        nc.sync.dma_start(
                            out=out_d[r0 + nfull * P:r0 + rows, :],
                            in_=outs[:rem, nfull, :])

            if repeat > 1:
                with tc.For_i(0, repeat, 1):
                    body()
            else:
                body()
    nc.compile()
    return nc


def make_in_maps(x, W, bias, st):
    x32 = np.asarray(x, np.float32)
    x_lo = x32[:SPLIT].astype(ml_dtypes.bfloat16)
    x_hi = np.ascontiguousarray(x32[SPLIT:]).astype(ml_dtypes.bfloat16)
    wt = np.ascontiguousarray(np.asarray(W, np.float32).T).astype(
        ml_dtypes.bfloat16)                                   # [i, o]
    iota = np.tile(np.arange(P, dtype=np.float32), (P, 1)).astype(
        ml_dtypes.bfloat16)
    bias_row = np.asarray(bias, np.float32)[None, :].astype(ml_dtypes.bfloat16)
    ones_row = np.ones((1, P), ml_dtypes.bfloat16)

    din_cols = st["din_flat"].reshape(C, st["K"], P).transpose(0, 2, 1)
    val_cols = st["val_flat"].reshape(C, st["K"], P).transpose(0, 2, 1)

    in_maps = []
    for c in range(C):
        in_maps.append({
            "x_lo": x_lo, "x_hi": x_hi,
            "idx": _wrap_idx(st["idx_flat"][c]),
            "din": np.ascontiguousarray(din_cols[c]),
            "val": np.ascontiguousarray(val_cols[c]),
            "wt": wt, "iota": iota, "bias_row": bias_row,
            "ones_row": ones_row,
        })
    return in_maps


def kernel(x, edge_row, edge_col, edge_val, W, bias):
    st = _prep_host(edge_row, edge_col, edge_val)
    nc = _build_program(st)
    in_maps = make_in_maps(x, W, bias, st)
    res = run_bass_kernel_spmd(nc, in_maps, core_ids=list(range(C)))
    out = np.concatenate([res.results[c]["out"] for c in range(C)], axis=0)
    return out.astype(np.float32)


if __name__ == "__main__":
    rng = np.random.default_rng(0)
    x = rng.standard_normal((N_NODES, D), dtype=np.float32)
    er = rng.integers(0, N_NODES, N_EDGES)
    ec = rng.integers(0, N_NODES, N_EDGES)
    ev = rng.random(N_EDGES, dtype=np.float32)
    W = rng.standard_normal((D, D), dtype=np.float32) / np.sqrt(D)
    b = np.zeros(D, np.float32)
    out = kernel(x, er, ec, ev, W, b)
    print(out.shape, out.dtype)

